# revision 1
# baseline (speedup 1.0000x reference)
"""Trainium2 Bass kernel for nn_AttentionNestedNERModel.

Strategy: data-parallel over batch (B=64 -> 8 cores x 8). Per core:
  phase 0: load weights, gather embeddings (indirect DMA), transpose to
           feature-major xT
  phase 1: precompute encoder input projections Zf/Zb as big matmuls
  phase 2: bidirectional encoder LSTM recurrence (128 steps, fwd+bwd
           interleaved in one loop); input projections pre-added
  phase 3: batch mid-phase: h_sb (token-major h), whT (attention weights),
           base0/base123 (decoder gate contributions that don't depend on
           the recurrence: W_h@h + W_e@x + W_p@prev_s + biases). Staged to
           DRAM so the encoder-phase SBUF pools can close (pools are a
           stack; lifetimes can't interleave).
  phase 4: decoder loop, 4 levels x 128 steps. Per step: attention scores
           via block-diagonal stationary trick -> softmax (exp with
           running-sum accum) -> context -> gate matmul (bf16 weights,
           fast-weight-load) -> LSTM cell math
  phase 5: (before level 1) reload base123 and fold in W_p @ level0-outputs
  phase 6: output projection to logits

All recurrent-loop matmul operands are bf16 (PSUM accumulation stays f32);
big precompute matmuls are f32.
"""

import sys

sys.path.insert(0, "/opt/trn_rl_repo")

import numpy as np
import ml_dtypes

import concourse.bass as bass
import concourse.mybir as mybir
import concourse.tile as tile
from concourse.masks import make_identity
from concourse.bass import ds

V, E, H, DH, LMAX, C = 25000, 512, 256, 512, 4, 9
B, S = 64, 128
NCORES = 8
Bc = B // NCORES            # 8 batch elements per core
NT = S * Bc                 # 1024 tokens per core, token index = t*Bc + b
F32 = mybir.dt.float32
BF16 = mybir.dt.bfloat16
U32 = mybir.dt.uint32
AX = mybir.AluOpType
AF = mybir.ActivationFunctionType
P = 128


def _split_sync_waits(nc, max_waits=1):
    """This walrus build rejects >1 sync wait on one instruction; split the
    excess onto same-engine NOPs placed immediately before."""
    n_split = 0
    for fn in nc.m.functions:
        for bb in fn.blocks:
            new_insts = []
            for inst in bb.instructions:
                si = inst.sync_info
                if si is not None and si.on_wait is not None and len(si.on_wait) > max_waits:
                    waits = list(si.on_wait)
                    keep = waits[-max_waits:]
                    rest = waits[:-max_waits]
                    for j in range(0, len(rest), max_waits):
                        nop = mybir.InstNoOp(
                            name=nc.get_next_instruction_name(),
                            engine=inst.engine,
                            ins=[], outs=[],
                            sync_info=mybir.SyncInfo(
                                on_wait=rest[j:j + max_waits], on_update=[]),
                        )
                        nc.register_instruction(nop)
                        new_insts.append(nop)
                    si.on_wait = keep
                    n_split += 1
                new_insts.append(inst)
            bb.instructions[:] = new_insts
    return n_split


def _r(dram, p=P):
    """[K, M] dram tensor -> [p, K//p, M] partition-major view."""
    return dram[:].rearrange("(kt p) m -> p kt m", p=p)


def build_nc(debug=False):
    import os as _os
    DEC_STEPS = int(_os.environ.get("DEC_STEPS", S))
    MERGED_STEPS = int(_os.environ.get("MERGED_STEPS", 3 * S))
    nc = bass.Bass()

    emb = nc.dram_tensor("emb", [V, E], F32, kind="ExternalInput")
    idx = nc.dram_tensor("idx", [S, Bc], U32, kind="ExternalInput")
    wihf = nc.dram_tensor("wihf", [E, 4 * H], F32, kind="ExternalInput")
    wihb = nc.dram_tensor("wihb", [E, 4 * H], F32, kind="ExternalInput")
    whhf = nc.dram_tensor("whhf", [H, 4 * H], BF16, kind="ExternalInput")
    whhb = nc.dram_tensor("whhb", [H, 4 * H], BF16, kind="ExternalInput")
    benc = nc.dram_tensor("benc", [P, 2, 8], F32, kind="ExternalInput")
    wlt = nc.dram_tensor("wlt", [DH, DH], F32, kind="ExternalInput")
    wcdt = nc.dram_tensor("wcdt", [2 * DH, 4 * DH], BF16, kind="ExternalInput")
    wat = nc.dram_tensor("wat", [2 * DH, 4 * DH], F32, kind="ExternalInput")
    wbt = nc.dram_tensor("wbt", [2 * DH, 4 * DH], F32, kind="ExternalInput")
    wpt = nc.dram_tensor("wpt", [DH, 4 * DH], BF16, kind="ExternalInput")
    bdec = nc.dram_tensor("bdec", [P, 16, 4], F32, kind="ExternalInput")
    w2t = nc.dram_tensor("w2t", [DH, C], BF16, kind="ExternalInput")
    b2v = nc.dram_tensor("b2v", [C, 1], F32, kind="ExternalInput")
    out = nc.dram_tensor("out", [LMAX, C, NT], F32, kind="ExternalOutput")

    # internal DRAM staging (cross-phase tensors; SBUF pools are a stack)
    whT_d = nc.dram_tensor("whT_d", [P, 4, Bc, S], BF16)
    hsb_d = nc.dram_tensor("hsb_d", [P, Bc, DH], BF16)
    b0_d = nc.dram_tensor("b0_d", [P, 16, NT], BF16)
    b123_d = nc.dram_tensor("b123_d", [P, 16, NT], BF16)

    dbg = {}
    if debug:
        dbg["xT"] = nc.dram_tensor("dbg_xT", [P, 4, NT], F32, kind="ExternalOutput")
        dbg["zfT"] = nc.dram_tensor("dbg_zfT", [P, 8, NT], F32, kind="ExternalOutput")
        dbg["hT"] = nc.dram_tensor("dbg_hT", [P, 4, NT], F32, kind="ExternalOutput")
        dbg["whT"] = nc.dram_tensor("dbg_whT", [P, 4, Bc, S], BF16, kind="ExternalOutput")
        dbg["base0"] = nc.dram_tensor("dbg_base0", [P, 16, NT], BF16, kind="ExternalOutput")
        dbg["outs"] = nc.dram_tensor("dbg_outs", [P, 4, LMAX * NT], BF16, kind="ExternalOutput")
        dbg["b123"] = nc.dram_tensor("dbg_b123", [P, 16, NT], BF16, kind="ExternalOutput")
        dbg["att"] = nc.dram_tensor("dbg_att", [Bc, S], F32, kind="ExternalOutput")
        dbg["ctx"] = nc.dram_tensor("dbg_ctx", [Bc, DH], F32, kind="ExternalOutput")
        dbg["g1"] = nc.dram_tensor("dbg_g1", [P, 16, Bc], F32, kind="ExternalOutput")
        dbg["hd"] = nc.dram_tensor("dbg_hd", [P, 4, Bc], F32, kind="ExternalOutput")

    with tile.TileContext(nc) as tc:
        with (
            tc.tile_pool(name="persist", bufs=1) as PT,
            tc.tile_pool(name="psbig", bufs=2, space="PSUM") as PSB,
        ):
            ident = PT.tile([P, P], F32)
            make_identity(nc, ident[:])
            bdec_sb = PT.tile([P, 16, 4], F32)
            nc.sync.dma_start(bdec_sb[:], bdec[:])
            w2t_sb = PT.tile([P, 4, C], BF16)
            nc.sync.dma_start(w2t_sb[:], _r(w2t))
            b2_sb = PT.tile([C, 1], F32)
            nc.sync.dma_start(b2_sb[:], b2v[:])

            with tc.tile_pool(name="ph03", bufs=1) as P03:
                xT = P03.tile([P, 4, NT], F32)
                hT = P03.tile([P, 4, NT], F32)
                wlt_sb = P03.tile([P, 4, DH], F32)

                with tc.tile_pool(name="phenc", bufs=1) as PE_:
                    zfT = PE_.tile([P, 8, NT], F32)
                    zbT = PE_.tile([P, 8, NT], F32)
                    whhf_sb = PE_.tile([P, 2, 4 * H], BF16)
                    whhb_sb = PE_.tile([P, 2, 4 * H], BF16)
                    benc_sb = PE_.tile([P, 2, 8], F32)
                    nc.sync.dma_start(whhf_sb[:], _r(whhf))
                    nc.sync.dma_start(whhb_sb[:], _r(whhb))
                    nc.sync.dma_start(benc_sb[:], benc[:])

                    # ------------- phase 0: gather + transpose -------------
                    with tc.tile_pool(name="ph01", bufs=1) as PA:
                        idx_sb = PA.tile([P, Bc], U32)
                        nc.sync.dma_start(idx_sb[:], idx[:])
                        wihf_sb = PA.tile([P, 4, 4 * H], F32)
                        nc.sync.dma_start(wihf_sb[:], _r(wihf))
                        wihb_sb = PA.tile([P, 4, 4 * H], F32)
                        nc.sync.dma_start(wihb_sb[:], _r(wihb))
                        nc.sync.dma_start(wlt_sb[:], _r(wlt))

                        x_sb = PA.tile([P, Bc, E], F32)
                        for b in range(Bc):
                            nc.gpsimd.indirect_dma_start(
                                out=x_sb[:, b, :],
                                out_offset=None,
                                in_=emb[:],
                                in_offset=bass.IndirectOffsetOnAxis(
                                    ap=idx_sb[:, b:b + 1], axis=0),
                                bounds_check=V - 1,
                                oob_is_err=False,
                            )

                        xT_r = xT[:].rearrange("p e (t b) -> p e t b", b=Bc)
                        for b in range(Bc):
                            for et in range(4):
                                pst = PSB.tile([P, 512], F32, tag="psbig")
                                nc.tensor.transpose(
                                    pst[:, :P], x_sb[:, b, et * P:(et + 1) * P], ident[:])
                                nc.vector.tensor_copy(out=xT_r[:, et, :, b], in_=pst[:, :P])

                        # ------------- phase 1: Zf / Zb -------------
                        for zT, wih_sb, dir_i in ((zfT, wihf_sb, 0), (zbT, wihb_sb, 1)):
                            for mt in range(8):
                                for nch in range(2):
                                    pst = PSB.tile([P, 512], F32, tag="psbig")
                                    for kt in range(4):
                                        nc.tensor.matmul(
                                            pst[:],
                                            lhsT=wih_sb[:, kt, mt * P:(mt + 1) * P],
                                            rhs=xT[:, kt, nch * 512:(nch + 1) * 512],
                                            start=(kt == 0), stop=(kt == 3),
                                        )
                                    nc.vector.tensor_tensor(
                                        out=zT[:, mt, nch * 512:(nch + 1) * 512],
                                        in0=pst[:],
                                        in1=benc_sb[:, dir_i, mt:mt + 1].to_broadcast([P, 512]),
                                        op=AX.add,
                                    )

                    # ------------- phase 2: encoder recurrence -------------
                    hf = PE_.tile([P, 2, Bc], F32)
                    cf = PE_.tile([P, 2, Bc], F32)
                    hb = PE_.tile([P, 2, Bc], F32)
                    cb = PE_.tile([P, 2, Bc], F32)
                    hf_bf = PE_.tile([P, 2, Bc], BF16)
                    hb_bf = PE_.tile([P, 2, Bc], BF16)
                    for t0 in (hf, cf, hb, cb, hf_bf, hb_bf):
                        nc.any.memset(t0[:], 0.0)
                    sig_e = PE_.tile([P, 2, 8, Bc], F32)
                    tmp_e = PE_.tile([P, 2, 6, Bc], F32)
                    g1_e = PE_.tile([P, 2, 8, Bc], F32)

                    ctx_pse = tc.tile_pool(name="psenc", bufs=2, space="PSUM")
                    PSE = ctx_pse.__enter__()
                    with tc.For_i(0, S, staggered_reset=True) as i:
                        for dir_i, (whh_sb, zT, h, c, h_bf, ht_lo) in enumerate((
                                (whhf_sb, zfT, hf, cf, hf_bf, 0),
                                (whhb_sb, zbT, hb, cb, hb_bf, 2))):
                            off = i * Bc if dir_i == 0 else (NT - Bc) - i * Bc
                            psg = PSE.tile([P, 8, Bc], F32, tag="psenc")
                            for mt in range(8):
                                for kt in range(2):
                                    nc.tensor.matmul(
                                        psg[:, mt, :],
                                        lhsT=whh_sb[:, kt, mt * P:(mt + 1) * P],
                                        rhs=h_bf[:, kt, :],
                                        start=(kt == 0), stop=(kt == 1),
                                    )
                            g1 = g1_e[:, dir_i]
                            nc.vector.tensor_tensor(
                                out=g1, in0=psg[:],
                                in1=zT[:, :, ds(off, Bc)], op=AX.add)
                            # all transcendentals via the exp_and_others table
                            # set: sigmoid(x) = 0.5 + 0.5*tanh(x/2)
                            sig = sig_e[:, dir_i]
                            nc.scalar.activation(sig, g1, AF.Tanh, scale=0.5)
                            nc.vector.tensor_scalar(sig[:, 0:4, :], sig[:, 0:4, :],
                                                    0.5, 0.5, AX.mult, AX.add)
                            nc.vector.tensor_scalar(sig[:, 6:8, :], sig[:, 6:8, :],
                                                    0.5, 0.5, AX.mult, AX.add)
                            si_ = sig[:, 0:2, :]
                            sf_ = sig[:, 2:4, :]
                            so_ = sig[:, 6:8, :]
                            tg = tmp_e[:, dir_i, 0:2, :]
                            t1 = tmp_e[:, dir_i, 2:4, :]
                            # tanh(g/2 * 2) -- g slice holds tanh(g/2)?? no:
                            # scale=0.5 gave tanh(g/2); need tanh(g): use the
                            # identity-free path: recompute tanh(g) exactly via
                            # a second ACT on the g slice only.
                            nc.scalar.activation(tg, g1[:, 4:6, :], AF.Tanh)
                            nc.vector.tensor_tensor(out=t1, in0=si_, in1=tg, op=AX.mult)
                            nc.vector.tensor_tensor(out=c[:], in0=sf_, in1=c[:], op=AX.mult)
                            nc.vector.tensor_tensor(out=c[:], in0=c[:], in1=t1, op=AX.add)
                            nc.scalar.activation(tg, c[:], AF.Tanh)
                            hslice = hT[:, ht_lo:ht_lo + 2, ds(off, Bc)]
                            nc.vector.tensor_tensor(out=h[:], in0=so_, in1=tg, op=AX.mult)
                            nc.scalar.copy(out=hslice, in_=h[:])
                            nc.vector.tensor_copy(out=h_bf[:], in_=h[:])

                    ctx_pse.__exit__(None, None, None)
                    if debug:
                        nc.sync.dma_start(dbg["zfT"][:], zfT[:])

                # ------------- phase 3: h_sb, whT, bases (staged to DRAM) ----
                with tc.tile_pool(name="ph3", bufs=1) as W3, \
                     tc.tile_pool(name="ph3st", bufs=2) as W3S:
                    h_sb3 = W3.tile([P, Bc, DH], BF16)
                    hT_r = hT[:].rearrange("p d (t b) -> p d t b", b=Bc)
                    for b in range(Bc):
                        for dt in range(4):
                            pst = PSB.tile([P, 512], F32, tag="psbig")
                            nc.tensor.transpose(pst[:, :P], hT_r[:, dt, :, b], ident[:])
                            nc.vector.tensor_copy(
                                out=h_sb3[:, b, dt * P:(dt + 1) * P], in_=pst[:, :P])
                    nc.sync.dma_start(hsb_d[:], h_sb3[:])

                    whT3 = W3.tile([P, 4, Bc, S], BF16)
                    for et in range(4):
                        for nch in range(2):
                            pst = PSB.tile([P, 512], F32, tag="psbig")
                            for kt in range(4):
                                nc.tensor.matmul(
                                    pst[:],
                                    lhsT=wlt_sb[:, kt, et * P:(et + 1) * P],
                                    rhs=hT[:, kt, nch * 512:(nch + 1) * 512],
                                    start=(kt == 0), stop=(kt == 3),
                                )
                            nc.vector.tensor_copy(
                                out=whT3[:, et, :, nch * 64:(nch + 1) * 64],
                                in_=pst[:].rearrange("p (t b) -> p b t", b=Bc),
                            )
                    nc.sync.dma_start(whT_d[:], whT3[:])

                    for b_dram, w_dram, bias_col in ((b0_d, wat, 0), (b123_d, wbt, None)):
                        base3 = W3.tile([P, 16, NT], BF16, tag="base3")
                        for mt2 in range(8):
                            wchunk = W3S.tile([P, 8, 2 * P], F32, tag="wchunk")
                            nc.sync.dma_start(
                                wchunk[:], _r(w_dram)[:, :, mt2 * 256:(mt2 + 1) * 256])
                            for mh in range(2):
                                mt = mt2 * 2 + mh
                                for nch in range(2):
                                    pst = PSB.tile([P, 512], F32, tag="psbig")
                                    for kt in range(8):
                                        rhs = (hT[:, kt, nch * 512:(nch + 1) * 512]
                                               if kt < 4 else
                                               xT[:, kt - 4, nch * 512:(nch + 1) * 512])
                                        nc.tensor.matmul(
                                            pst[:],
                                            lhsT=wchunk[:, kt, mh * P:(mh + 1) * P],
                                            rhs=rhs,
                                            start=(kt == 0), stop=(kt == 7),
                                        )
                                    if bias_col is None:
                                        nc.vector.tensor_copy(
                                            out=base3[:, mt, nch * 512:(nch + 1) * 512],
                                            in_=pst[:])
                                    else:
                                        nc.vector.tensor_tensor(
                                            out=base3[:, mt, nch * 512:(nch + 1) * 512],
                                            in0=pst[:],
                                            in1=bdec_sb[:, mt, bias_col:bias_col + 1]
                                            .to_broadcast([P, 512]),
                                            op=AX.add,
                                        )
                        nc.sync.dma_start(b_dram[:], base3[:])

                    if debug:
                        nc.sync.dma_start(dbg["xT"][:], xT[:])
                        nc.sync.dma_start(dbg["hT"][:], hT[:])
                        nc.sync.dma_start(dbg["whT"][:], whT3[:])

            # ---------------- phase 4: decoder ----------------
            with tc.tile_pool(name="pdec", bufs=1) as PD, \
                 tc.tile_pool(name="pdecst", bufs=2) as PDS, \
                 tc.tile_pool(name="psdec", bufs=1, space="PSUM") as PSD, \
                 tc.tile_pool(name="pssmall", bufs=1, space="PSUM") as PSS:
                wcdt_sb = PD.tile([P, 8, 4 * DH], BF16)
                nc.sync.dma_start(wcdt_sb[:], _r(wcdt))
                h_sb = PD.tile([P, Bc, DH], BF16)
                nc.sync.dma_start(h_sb[:], hsb_d[:])
                whT = PD.tile([P, 4, Bc, S], BF16)
                nc.sync.dma_start(whT[:], whT_d[:])
                base_sb = PD.tile([P, 16, NT], BF16)
                nc.sync.dma_start(base_sb[:], b0_d[:])

                outs = PD.tile([P, 4, LMAX * NT], BF16)
                if DEC_STEPS != S or MERGED_STEPS != 3 * S:
                    nc.any.memset(outs[:], 0.0)
                cd = PD.tile([P, 4, Bc], F32)
                hd_bf = PD.tile([P, 4, Bc], BF16)
                diag_at = PD.tile([P, Bc, Bc], BF16)
                ones_col = PD.tile([P, 1], F32)
                nc.any.memset(cd[:], 0.0)
                nc.any.memset(hd_bf[:], 0.0)
                nc.any.memset(diag_at[:], 0.0)
                nc.any.memset(ones_col[:], 1.0)
                diag_at_v = diag_at[:].rearrange("p a b -> p (a b)")[:, 0:64:9]

                sigd = PD.tile([P, 16, Bc], F32)
                tmpd = PD.tile([P, 3, 4, Bc], F32)
                g1_d = PD.tile([P, 16, Bc], F32)
                g1a_d = PD.tile([P, 16, Bc], F32)
                att_eT = PD.tile([S, Bc], F32)
                rz = PD.tile([Bc, 1], F32)
                ctx_sb = PD.tile([Bc, DH], F32)
                ctxT_bf = PD.tile([P, 4, Bc], BF16)

                def dec_step(base_off, outs_off, bias_ix):
                    # scores, transposed: ps_scT[s, b] = sum_d whT[d,b,s]*hd[d,b]
                    # (whT tile is the stationary operand; hd column streams).
                    ps_scT = PSD.tile([S, Bc], F32, tag="ps_sc")
                    for b in range(Bc):
                        for dt in range(4):
                            nc.tensor.matmul(
                                ps_scT[:, b:b + 1],
                                lhsT=whT[:, dt, b, :],
                                rhs=hd_bf[:, dt, b:b + 1],
                                start=(dt == 0), stop=(dt == 3),
                            )
                    # gates, hd half (kt 4..7) can start immediately.
                    # Per-mt accumulation groups must be contiguous: interleaved
                    # start=True groups in one psum bank corrupt accumulation,
                    # so the hd half and ctx half use separate psum tiles.
                    ps_g = PSD.tile([P, 16, Bc], F32, tag="ps_g")
                    for mt in range(16):
                        for kt in range(4, 8):
                            nc.tensor.matmul(
                                ps_g[:, mt, :],
                                lhsT=wcdt_sb[:, kt, mt * P:(mt + 1) * P],
                                rhs=hd_bf[:, kt - 4, :],
                                start=(kt == 4), stop=(kt == 7),
                            )
                    # softmax pieces (|scores| < ~1, so no max-subtraction
                    # needed); att lands s-on-partitions, so the partition
                    # reduction for Z is a ones-vector matmul.
                    nc.scalar.activation(att_eT[:], ps_scT[:], AF.Exp)
                    ps_z = PSS.tile([Bc, 1], F32, tag="ps_z")
                    nc.tensor.matmul(ps_z[:], lhsT=att_eT[:], rhs=ones_col[:],
                                     start=True, stop=True)
                    nc.vector.reciprocal(rz[:], ps_z[:])
                    nc.vector.tensor_copy(out=diag_at_v, in_=att_eT[:])
                    # ctx: ps_ctx[b, :] += att_b . h_sb[:, b, :]
                    ps_ctx = PSS.tile([Bc, DH], F32, tag="ps_ctx")
                    for b in range(Bc):
                        nc.tensor.matmul(
                            ps_ctx[:],
                            lhsT=diag_at[:, b, :],
                            rhs=h_sb[:, b, :],
                            start=(b == 0), stop=(b == Bc - 1),
                        )
                    # normalize while evacuating
                    nc.vector.tensor_tensor(
                        out=ctx_sb[:], in0=ps_ctx[:],
                        in1=rz[:].to_broadcast([Bc, DH]), op=AX.mult)
                    # transpose ctx to feature-major
                    ps_ct = PSS.tile([P, 4 * Bc], F32, tag="ps_ct")
                    for dt in range(4):
                        nc.tensor.transpose(
                            ps_ct[:, dt * Bc:(dt + 1) * Bc],
                            ctx_sb[:, dt * P:(dt + 1) * P], ident[:Bc, :Bc])
                    nc.vector.tensor_copy(
                        out=ctxT_bf[:].rearrange("p d b -> p (d b)"), in_=ps_ct[:])
                    # fold base into the hd-half early (off the critical path)
                    nc.vector.tensor_tensor(
                        out=g1a_d[:], in0=ps_g[:],
                        in1=base_sb[:, :, ds(base_off, Bc)], op=AX.add)
                    if bias_ix is not None:
                        nc.vector.tensor_tensor(
                            out=g1a_d[:], in0=g1a_d[:],
                            in1=bdec_sb[:, :, ds(bias_ix, 1)].to_broadcast([P, 16, Bc]),
                            op=AX.add)
                    # gates, ctx half (kt 0..3) into its own psum tile
                    ps_g2 = PSD.tile([P, 16, Bc], F32, tag="ps_g2")
                    for mt in range(16):
                        for kt in range(4):
                            nc.tensor.matmul(
                                ps_g2[:, mt, :],
                                lhsT=wcdt_sb[:, kt, mt * P:(mt + 1) * P],
                                rhs=ctxT_bf[:, kt, :],
                                start=(kt == 0), stop=(kt == 3),
                            )
                    # cell math
                    nc.vector.tensor_tensor(
                        out=g1_d[:], in0=g1a_d[:], in1=ps_g2[:], op=AX.add)
                    nc.scalar.activation(sigd[:], g1_d[:], AF.Tanh, scale=0.5)
                    nc.vector.tensor_scalar(sigd[:, 0:8, :], sigd[:, 0:8, :],
                                            0.5, 0.5, AX.mult, AX.add)
                    nc.vector.tensor_scalar(sigd[:, 12:16, :], sigd[:, 12:16, :],
                                            0.5, 0.5, AX.mult, AX.add)
                    si_ = sigd[:, 0:4, :]
                    sf_ = sigd[:, 4:8, :]
                    so_ = sigd[:, 12:16, :]
                    tg = tmpd[:, 0]
                    t1 = tmpd[:, 1]
                    nc.scalar.activation(tg, g1_d[:, 8:12, :], AF.Tanh)
                    nc.vector.tensor_tensor(out=t1, in0=si_, in1=tg, op=AX.mult)
                    nc.vector.tensor_tensor(out=cd[:], in0=sf_, in1=cd[:], op=AX.mult)
                    nc.vector.tensor_tensor(out=cd[:], in0=cd[:], in1=t1, op=AX.add)
                    nc.scalar.activation(tg, cd[:], AF.Tanh)
                    nc.vector.tensor_tensor(out=hd_bf[:], in0=so_, in1=tg, op=AX.mult)
                    nc.scalar.copy(out=outs[:, :, ds(outs_off, Bc)], in_=hd_bf[:])

                # level 0
                with tc.For_i(0, DEC_STEPS, hint_engines=(mybir.EngineType.PE,), staggered_reset=True) as i:
                    off0 = i * Bc
                    dec_step(off0, off0, None)

                if debug:
                    nc.sync.dma_start(dbg["base0"][:], base_sb[:])
                # reload base123, then fold in W_p @ outs[level 0]
                nc.sync.dma_start(base_sb[:], b123_d[:])
                for mt2 in range(8):
                    wpchunk = PDS.tile([P, 4, 2 * P], BF16, tag="wpchunk")
                    nc.sync.dma_start(
                        wpchunk[:], _r(wpt)[:, :, mt2 * 256:(mt2 + 1) * 256])
                    for mh in range(2):
                        mt = mt2 * 2 + mh
                        for nch in range(2):
                            pst = PSB.tile([P, 512], F32, tag="psbig")
                            for kt in range(4):
                                nc.tensor.matmul(
                                    pst[:],
                                    lhsT=wpchunk[:, kt, mh * P:(mh + 1) * P],
                                    rhs=outs[:, kt, nch * 512:(nch + 1) * 512],
                                    start=(kt == 0), stop=(kt == 3),
                                )
                            bslice = base_sb[:, mt, nch * 512:(nch + 1) * 512]
                            nc.vector.tensor_tensor(
                                out=bslice, in0=bslice, in1=pst[:], op=AX.add)

                if debug:
                    nc.sync.dma_start(dbg["b123"][:], base_sb[:])
                # levels 1..3 merged: t = i % S, lvl = i // S + 1
                with tc.For_i(0, MERGED_STEPS, hint_engines=(mybir.EngineType.PE,), staggered_reset=True) as i:
                    toff = (i % S) * Bc
                    ooff = NT + i * Bc
                    lv = i // S + 1
                    dec_step(toff, ooff, lv)

                # ---------------- phase 6: logits ----------------
                for lvl in range(LMAX):
                    lg = PDS.tile([C, NT], F32, tag="lg")
                    for nch in range(2):
                        ps_lg = PSB.tile([P, 512], F32, tag="psbig")
                        for kt in range(4):
                            nc.tensor.matmul(
                                ps_lg[:C, :],
                                lhsT=w2t_sb[:, kt, :],
                                rhs=outs[:, kt,
                                         lvl * NT + nch * 512:lvl * NT + (nch + 1) * 512],
                                start=(kt == 0), stop=(kt == 3),
                            )
                        nc.vector.tensor_tensor(
                            out=lg[:, nch * 512:(nch + 1) * 512],
                            in0=ps_lg[:C, :],
                            in1=b2_sb[:].to_broadcast([C, 512]),
                            op=AX.add,
                        )
                    nc.sync.dma_start(out[lvl], lg[:])

                if debug:
                    nc.sync.dma_start(dbg["outs"][:], outs[:])
                    pass  # dbg att dropped (layout changed to att_eT)
                    nc.sync.dma_start(dbg["ctx"][:], ctx_sb[:])
                    nc.sync.dma_start(dbg["g1"][:], g1_d[:])
                    dbg_hd_f = PDS.tile([P, 4, Bc], F32, tag="dbghd")
                    nc.vector.tensor_copy(out=dbg_hd_f[:], in_=hd_bf[:])
                    nc.sync.dma_start(dbg["hd"][:], dbg_hd_f[:])

    _split_sync_waits(nc, max_waits=1)
    return nc


def _gate_scale(w, lo, hi):
    w = np.array(w, dtype=np.float32, copy=True)
    w[lo:hi] *= 2.0
    return w


def host_prep(inputs):
    """Build the per-core in_maps from the full problem inputs."""
    f32 = lambda a: np.ascontiguousarray(np.asarray(a, dtype=np.float32))
    bf16 = lambda a: np.ascontiguousarray(
        np.asarray(a, dtype=np.float32).astype(ml_dtypes.bfloat16))

    seqs = np.asarray(inputs["seqs"])
    emb = f32(inputs["emb"])

    # encoder weights: i,f,g,o gate order; scale g rows (2H..3H) by 2
    def enc_prep(wih, whh, bih, bhh):
        wih = f32(inputs[wih])
        whh = f32(inputs[whh])
        bias = f32(inputs[bih]) + f32(inputs[bhh])
        return wih.T.copy(), whh.T.copy(), bias

    wihf_t, whhf_t, bf_ = enc_prep("Wih_f", "Whh_f", "bih_f", "bhh_f")
    wihb_t, whhb_t, bb_ = enc_prep("Wih_b", "Whh_b", "bih_b", "bhh_b")
    benc = np.stack([bf_.reshape(8, P).T, bb_.reshape(8, P).T], axis=1)  # [p, dir, mt]

    wl_t = f32(inputs["Wl"]).T.copy()

    wih_d = f32(inputs["Wih_d"])
    whh_d = f32(inputs["Whh_d"])
    bd = f32(inputs["bih_d"]) + f32(inputs["bhh_d"])
    w_ctx = wih_d[:, 0:DH]
    w_h = wih_d[:, DH:2 * DH]
    w_e = wih_d[:, 2 * DH:3 * DH]
    w_p = wih_d[:, 3 * DH:4 * DH]
    w_oh = wih_d[:, 4 * DH:4 * DH + LMAX]

    wcd_t = np.concatenate([w_ctx, whh_d], axis=1).T.copy()        # [1024, 2048]
    wa_t = np.concatenate([w_h + w_p, w_e], axis=1).T.copy()       # [1024, 2048]
    wb_t = np.concatenate([w_h, w_e], axis=1).T.copy()             # [1024, 2048]
    wp_t = w_p.T.copy()                                            # [512, 2048]

    bias_l = bd[None, :] + w_oh.T                                  # [4, 2048]
    bcols = bias_l.T.copy()                                        # [2048, 4]
    bdec = bcols.reshape(16, P, 4).transpose(1, 0, 2).copy()       # [p, mt, col]

    w2_t = f32(inputs["W2"]).T.copy()
    b2v = f32(inputs["b2"]).reshape(C, 1)

    shared = {
        "emb": emb,
        "wihf": f32(wihf_t), "wihb": f32(wihb_t),
        "whhf": bf16(whhf_t), "whhb": bf16(whhb_t),
        "benc": f32(benc),
        "wlt": f32(wl_t),
        "wcdt": bf16(wcd_t),
        "wat": f32(wa_t), "wbt": f32(wb_t),
        "wpt": bf16(wp_t),
        "bdec": f32(bdec),
        "w2t": bf16(w2_t),
        "b2v": b2v,
    }
    in_maps = []
    for c in range(NCORES):
        m = dict(shared)
        m["idx"] = np.ascontiguousarray(
            seqs[c * Bc:(c + 1) * Bc].T.astype(np.uint32))          # [S, Bc]
        in_maps.append(m)
    return in_maps


_NC_CACHE = {}


def get_nc(debug=False):
    if debug not in _NC_CACHE:
        _NC_CACHE[debug] = build_nc(debug)
    return _NC_CACHE[debug]


def kernel(**inputs):
    from concourse.bass_utils import run_bass_kernel_spmd

    nc = get_nc(debug=False)
    in_maps = host_prep(inputs)
    res = run_bass_kernel_spmd(nc, in_maps, core_ids=list(range(NCORES)))
    lvl = int(np.asarray(inputs["seq_max_nested_level"]))
    lvl = max(1, min(LMAX, lvl))
    # out per core: [LMAX, C, NT] with token = t*Bc + b
    full = np.empty((LMAX, S, B, C), dtype=np.float32)
    for c in range(NCORES):
        o = np.asarray(res.results[c]["out"])
        full[:, :, c * Bc:(c + 1) * Bc, :] = (
            o.transpose(0, 2, 1).reshape(LMAX, S, Bc, C))
    return full[:lvl].reshape(-1, C)



# revision 22
# speedup vs baseline: 1.7300x; 1.7300x over previous
"""Trainium2 Bass kernel for nn_AttentionNestedNERModel.

Strategy: data-parallel over batch (B=64 -> 8 cores x 8). Per core:
  phase 0: load weights, gather embeddings (indirect DMA), transpose to
           feature-major xT (bf16)
  phase 1: precompute encoder input projections Zf/Zb as big bf16 matmuls
  phase 2: bidirectional encoder LSTM recurrence (128 steps, fwd+bwd
           interleaved); cell math in the doubled basis (H=2h, C=2c) so
           sigmoid(x)=0.5(1+tanh(x/2)) folds into scalar_tensor_tensor ops
  phase 3: whT (attention weights), G = h @ W_ctx (context gate projection
           pre-applied to every encoder state), base0/base123 (decoder gate
           contributions independent of the recurrence)
  phase 4: decoder loop, 4 levels x 128 steps. Per step: attention scores
           (whT stationary) -> exp -> G matvec gives ctx-half gates directly
           -> 1/Z normalization via ones-matmul partition broadcast ->
           doubled-basis LSTM cell
  phase 5: (before level 1) reload base123, fold in W_p @ level0-outputs
  phase 6: output projection to logits

All steady-state matmul operands are bf16; fp32 only for cell state and
gate accumulation. Host pre-scales weights for the doubled basis (x0.5 on
inputs consuming h/hd, x0.25 on Wl) and the tanh-gate trick (x2 on g-gate
outputs), and reorders gates i,f,g,o -> i,f,o,g so the sigmoid gates are
contiguous.
"""

import sys

sys.path.insert(0, "/opt/trn_rl_repo")

import numpy as np
import ml_dtypes

import concourse.bass as bass
import concourse.mybir as mybir
import concourse.tile as tile
from concourse.masks import make_identity
from concourse.bass import ds

V, E, H, DH, LMAX, C = 25000, 512, 256, 512, 4, 9
B, S = 64, 128
NCORES = 8
Bc = B // NCORES            # 8 batch elements per core
NT = S * Bc                 # 1024 tokens per core, token index = t*Bc + b
F32 = mybir.dt.float32
BF16 = mybir.dt.bfloat16
U32 = mybir.dt.uint32
AX = mybir.AluOpType
AF = mybir.ActivationFunctionType
P = 128


def _split_sync_waits(nc, max_waits=1):
    """This walrus build rejects >1 sync wait on one instruction; split the
    excess onto same-engine NOPs placed immediately before."""
    n_split = 0
    for fn in nc.m.functions:
        for bb in fn.blocks:
            new_insts = []
            for inst in bb.instructions:
                si = inst.sync_info
                if si is not None and si.on_wait is not None and len(si.on_wait) > max_waits:
                    waits = list(si.on_wait)
                    keep = waits[-max_waits:]
                    rest = waits[:-max_waits]
                    for j in range(0, len(rest), max_waits):
                        nop = mybir.InstNoOp(
                            name=nc.get_next_instruction_name(),
                            engine=inst.engine,
                            ins=[], outs=[],
                            sync_info=mybir.SyncInfo(
                                on_wait=rest[j:j + max_waits], on_update=[]),
                        )
                        nc.register_instruction(nop)
                        new_insts.append(nop)
                    si.on_wait = keep
                    n_split += 1
                new_insts.append(inst)
            bb.instructions[:] = new_insts
    return n_split


def _r(dram, p=P):
    """[K, M] dram tensor -> [p, K//p, M] partition-major view."""
    return dram[:].rearrange("(kt p) m -> p kt m", p=p)


def build_nc(debug=False):
    import os as _os
    DEC_STEPS = int(_os.environ.get("DEC_STEPS", S))
    MERGED_STEPS = int(_os.environ.get("MERGED_STEPS", 3 * S))
    nc = bass.Bass()

    emb = nc.dram_tensor("emb", [V, E], F32, kind="ExternalInput")
    idx = nc.dram_tensor("idx", [S, Bc], U32, kind="ExternalInput")
    wihf = nc.dram_tensor("wihf", [E, 4 * H], BF16, kind="ExternalInput")
    wihb = nc.dram_tensor("wihb", [E, 4 * H], BF16, kind="ExternalInput")
    whhf = nc.dram_tensor("whhf", [H, 4 * H], BF16, kind="ExternalInput")
    whhb = nc.dram_tensor("whhb", [H, 4 * H], BF16, kind="ExternalInput")
    benc = nc.dram_tensor("benc", [P, 2, 8], F32, kind="ExternalInput")
    wlt = nc.dram_tensor("wlt", [DH, DH], BF16, kind="ExternalInput")
    wcdt = nc.dram_tensor("wcdt", [DH, 4 * DH], BF16, kind="ExternalInput")
    wct = nc.dram_tensor("wct", [DH, 4 * DH], BF16, kind="ExternalInput")
    wat = nc.dram_tensor("wat", [2 * DH, 4 * DH], BF16, kind="ExternalInput")
    wbt = nc.dram_tensor("wbt", [2 * DH, 4 * DH], BF16, kind="ExternalInput")
    wpt = nc.dram_tensor("wpt", [DH, 4 * DH], BF16, kind="ExternalInput")
    bdec = nc.dram_tensor("bdec", [P, 16, 4], F32, kind="ExternalInput")
    brow = nc.dram_tensor("brow", [1, LMAX * 4 * DH], BF16, kind="ExternalInput")
    w2t = nc.dram_tensor("w2t", [DH, C], BF16, kind="ExternalInput")
    b2v = nc.dram_tensor("b2v", [C, 1], F32, kind="ExternalInput")
    out = nc.dram_tensor("out", [LMAX, C, NT], F32, kind="ExternalOutput")

    # internal DRAM staging (cross-phase tensors; SBUF pools are a stack)
    whT_d = nc.dram_tensor("whT_d", [P, 4, Bc, S], BF16)
    b0_d = nc.dram_tensor("b0_d", [P, 16, NT], BF16)
    b123_d = nc.dram_tensor("b123_d", [P, 16, NT], BF16)

    with tile.TileContext(nc) as tc:
        with (
            tc.tile_pool(name="persist", bufs=1) as PT,
            tc.tile_pool(name="psbig", bufs=2, space="PSUM") as PSB,
        ):
            ident = PT.tile([P, P], F32)
            make_identity(nc, ident[:])
            bdec_sb = PT.tile([P, 16, 4], F32)
            nc.sync.dma_start(bdec_sb[:], bdec[:])
            w2t_sb = PT.tile([P, 4, C], BF16)
            nc.sync.dma_start(w2t_sb[:], _r(w2t))
            b2_sb = PT.tile([C, 1], F32)
            nc.sync.dma_start(b2_sb[:], b2v[:])
            brow_sb = PT.tile([1, LMAX * 4 * DH], BF16)
            nc.sync.dma_start(brow_sb[:], brow[:])
            G_lay = PT.tile([P, Bc, 4 * DH], BF16)   # [s, b, m] ctx gate proj
            ones_col = PT.tile([S, 1], BF16)
            ones1 = PT.tile([1, P], BF16)
            nc.any.memset(ones_col[:], 1.0)
            nc.any.memset(ones1[:], 1.0)

            with tc.tile_pool(name="ph03", bufs=1) as P03:
                xT = P03.tile([P, 4, NT], BF16)
                hT = P03.tile([P, 4, NT], BF16)
                wlt_sb = P03.tile([P, 4, DH], BF16)

                with tc.tile_pool(name="phenc", bufs=1) as PE_:
                    zfT = PE_.tile([P, 8, NT], BF16)
                    zbT = PE_.tile([P, 8, NT], BF16)
                    whhf_sb = PE_.tile([P, 2, 4 * H], BF16)
                    whhb_sb = PE_.tile([P, 2, 4 * H], BF16)
                    benc_sb = PE_.tile([P, 2, 8], F32)
                    nc.sync.dma_start(whhf_sb[:], _r(whhf))
                    nc.sync.dma_start(whhb_sb[:], _r(whhb))
                    nc.sync.dma_start(benc_sb[:], benc[:])

                    # ------------- phase 0: gather + transpose -------------
                    with tc.tile_pool(name="ph01", bufs=1) as PA:
                        idx_sb = PA.tile([P, Bc], U32)
                        nc.sync.dma_start(idx_sb[:], idx[:])
                        wihf_sb = PA.tile([P, 4, 4 * H], BF16)
                        nc.sync.dma_start(wihf_sb[:], _r(wihf))
                        wihb_sb = PA.tile([P, 4, 4 * H], BF16)
                        nc.sync.dma_start(wihb_sb[:], _r(wihb))
                        nc.sync.dma_start(wlt_sb[:], _r(wlt))

                        x_sb = PA.tile([P, Bc, E], F32)
                        for b in range(Bc):
                            nc.gpsimd.indirect_dma_start(
                                out=x_sb[:, b, :],
                                out_offset=None,
                                in_=emb[:],
                                in_offset=bass.IndirectOffsetOnAxis(
                                    ap=idx_sb[:, b:b + 1], axis=0),
                                bounds_check=V - 1,
                                oob_is_err=False,
                            )

                        xT_r = xT[:].rearrange("p e (t b) -> p e t b", b=Bc)
                        for b in range(Bc):
                            for et in range(4):
                                pst = PSB.tile([P, 512], F32, tag="psbig")
                                nc.tensor.transpose(
                                    pst[:, :P], x_sb[:, b, et * P:(et + 1) * P], ident[:])
                                nc.vector.tensor_copy(out=xT_r[:, et, :, b], in_=pst[:, :P])

                        # ------------- phase 1: Zf / Zb -------------
                        for zT, wih_sb, dir_i in ((zfT, wihf_sb, 0), (zbT, wihb_sb, 1)):
                            for mt in range(8):
                                for nch in range(2):
                                    pst = PSB.tile([P, 512], F32, tag="psbig")
                                    for kt in range(4):
                                        nc.tensor.matmul(
                                            pst[:],
                                            lhsT=wih_sb[:, kt, mt * P:(mt + 1) * P],
                                            rhs=xT[:, kt, nch * 512:(nch + 1) * 512],
                                            start=(kt == 0), stop=(kt == 3),
                                        )
                                    nc.vector.tensor_tensor(
                                        out=zT[:, mt, nch * 512:(nch + 1) * 512],
                                        in0=pst[:],
                                        in1=benc_sb[:, dir_i, mt:mt + 1].to_broadcast([P, 512]),
                                        op=AX.add,
                                    )

                    # ------------- phase 2: encoder recurrence -------------
                    # doubled basis: h_bf holds 2h, c holds 2c. Gate tiles
                    # (reordered i,f,o,g): tau_x = tanh(pre/2) for i,f,o and
                    # tanh(pre) for g (host pre-scaled g rows by 2).
                    # Both directions fused into one set of wide cell ops;
                    # dir is dim 1 of every state tile.
                    c2 = PE_.tile([P, 2, 2, Bc], F32)
                    h2_bf = PE_.tile([P, 2, 2, Bc], BF16)
                    nc.any.memset(c2[:], 0.0)
                    nc.any.memset(h2_bf[:], 0.0)
                    sig_e = PE_.tile([P, 2, 8, Bc], F32)
                    g1_e = PE_.tile([P, 2, 8, Bc], F32)
                    A2 = PE_.tile([P, 2, 2, Bc], F32)
                    B2 = PE_.tile([P, 2, 2, Bc], F32)
                    tc2 = PE_.tile([P, 2, 2, Bc], F32)

                    EU = int(_os.environ.get("EU", 1))
                    ctx_pse = tc.tile_pool(name="psenc", bufs=2, space="PSUM")
                    PSE = ctx_pse.__enter__()
                    with tc.For_i(0, S // EU, staggered_reset=True) as ii:
                        for u in range(EU):
                            if u > 0:
                                tc.stage_boundary()
                            psg = PSE.tile([P, 2, 8, Bc], F32, tag="psenc")
                            for dir_i, whh_sb in enumerate((whhf_sb, whhb_sb)):
                                for mt in range(8):
                                    for kt in range(2):
                                        nc.tensor.matmul(
                                            psg[:, dir_i, mt, :],
                                            lhsT=whh_sb[:, kt, mt * P:(mt + 1) * P],
                                            rhs=h2_bf[:, dir_i, kt, :],
                                            start=(kt == 0), stop=(kt == 1),
                                        )
                            off0 = ii * (EU * Bc) + u * Bc
                            off1 = ii * (-EU * Bc) + (NT - Bc - u * Bc)
                            nc.vector.tensor_tensor(
                                out=g1_e[:, 0], in0=psg[:, 0],
                                in1=zfT[:, :, ds(off0, Bc)], op=AX.add)
                            nc.vector.tensor_tensor(
                                out=g1_e[:, 1], in0=psg[:, 1],
                                in1=zbT[:, :, ds(off1, Bc)], op=AX.add)
                            nc.scalar.activation(sig_e[:], g1_e[:], AF.Tanh, scale=0.5)
                            # A = (tau_i+1)*tg ; B = (tau_f+1)*C ;
                            # C' = 0.5*B + A ; tc = tanh(C'/2) ; H = (tau_o+1)*tc
                            nc.vector.scalar_tensor_tensor(
                                out=A2[:], in0=sig_e[:, :, 0:2, :], scalar=1.0,
                                in1=sig_e[:, :, 6:8, :], op0=AX.add, op1=AX.mult)
                            nc.vector.scalar_tensor_tensor(
                                out=B2[:], in0=sig_e[:, :, 2:4, :], scalar=1.0,
                                in1=c2[:], op0=AX.add, op1=AX.mult)
                            nc.vector.scalar_tensor_tensor(
                                out=c2[:], in0=B2[:], scalar=0.5,
                                in1=A2[:], op0=AX.mult, op1=AX.add)
                            nc.scalar.activation(tc2[:], c2[:], AF.Tanh, scale=0.5)
                            nc.vector.scalar_tensor_tensor(
                                out=h2_bf[:], in0=sig_e[:, :, 4:6, :], scalar=1.0,
                                in1=tc2[:], op0=AX.add, op1=AX.mult)
                            nc.scalar.copy(
                                out=hT[:, 0:2, ds(off0, Bc)], in_=h2_bf[:, 0])
                            nc.scalar.copy(
                                out=hT[:, 2:4, ds(off1, Bc)], in_=h2_bf[:, 1])
                        for _ in range(3 - (EU - 1)):
                            tc.stage_boundary()

                    ctx_pse.__exit__(None, None, None)

                # ------------- phase 3: whT, G, bases (staged to DRAM) ----
                with tc.tile_pool(name="ph3", bufs=1) as W3, \
                     tc.tile_pool(name="ph3st", bufs=2) as W3S:
                    whT3 = W3.tile([P, 4, Bc, S], BF16)
                    for et in range(4):
                        for nch in range(2):
                            pst = PSB.tile([P, 512], F32, tag="psbig")
                            for kt in range(4):
                                nc.tensor.matmul(
                                    pst[:],
                                    lhsT=wlt_sb[:, kt, et * P:(et + 1) * P],
                                    rhs=hT[:, kt, nch * 512:(nch + 1) * 512],
                                    start=(kt == 0), stop=(kt == 3),
                                )
                            nc.vector.tensor_copy(
                                out=whT3[:, et, :, nch * 64:(nch + 1) * 64],
                                in_=pst[:].rearrange("p (t b) -> p b t", b=Bc),
                            )
                    nc.sync.dma_start(whT_d[:], whT3[:])

                    # G = per-token ctx gate projection: G[s,b,m] = h[s,b,:]@Wc
                    wct_sb = W3.tile([P, 4, 4 * DH], BF16)
                    nc.sync.dma_start(wct_sb[:], _r(wct))
                    hT_r = hT[:].rearrange("p k (t b) -> p k t b", b=Bc)
                    for b in range(Bc):
                        for mch in range(4):
                            pst = PSB.tile([P, 512], F32, tag="psbig")
                            for kt in range(4):
                                nc.tensor.matmul(
                                    pst[:],
                                    lhsT=hT_r[:, kt, :, b],
                                    rhs=wct_sb[:, kt, mch * 512:(mch + 1) * 512],
                                    start=(kt == 0), stop=(kt == 3),
                                )
                            nc.vector.tensor_copy(
                                out=G_lay[:, b, mch * 512:(mch + 1) * 512],
                                in_=pst[:])

                    for b_dram, w_dram, bias_col in ((b0_d, wat, 0), (b123_d, wbt, None)):
                        base3 = W3.tile([P, 16, NT], BF16, tag="base3")
                        for mt2 in range(8):
                            wchunk = W3S.tile([P, 8, 2 * P], BF16, tag="wchunk")
                            nc.sync.dma_start(
                                wchunk[:], _r(w_dram)[:, :, mt2 * 256:(mt2 + 1) * 256])
                            for mh in range(2):
                                mt = mt2 * 2 + mh
                                for nch in range(2):
                                    pst = PSB.tile([P, 512], F32, tag="psbig")
                                    for kt in range(8):
                                        rhs = (hT[:, kt, nch * 512:(nch + 1) * 512]
                                               if kt < 4 else
                                               xT[:, kt - 4, nch * 512:(nch + 1) * 512])
                                        nc.tensor.matmul(
                                            pst[:],
                                            lhsT=wchunk[:, kt, mh * P:(mh + 1) * P],
                                            rhs=rhs,
                                            start=(kt == 0), stop=(kt == 7),
                                        )
                                    if bias_col is None:
                                        nc.vector.tensor_copy(
                                            out=base3[:, mt, nch * 512:(nch + 1) * 512],
                                            in_=pst[:])
                                    else:
                                        nc.vector.tensor_tensor(
                                            out=base3[:, mt, nch * 512:(nch + 1) * 512],
                                            in0=pst[:],
                                            in1=bdec_sb[:, mt, bias_col:bias_col + 1]
                                            .to_broadcast([P, 512]),
                                            op=AX.add,
                                        )
                        nc.sync.dma_start(b_dram[:], base3[:])

            # ---------------- phase 4: decoder ----------------
            with tc.tile_pool(name="pdec", bufs=1) as PD, \
                 tc.tile_pool(name="pdecst", bufs=2) as PDS, \
                 tc.tile_pool(name="psdec", bufs=1, space="PSUM") as PSD:
                wcdt_sb = PD.tile([P, 4, 4 * DH], BF16)
                nc.sync.dma_start(wcdt_sb[:], _r(wcdt))
                whT = PD.tile([P, 4, Bc, S], BF16)
                nc.sync.dma_start(whT[:], whT_d[:])
                base0_sb = PD.tile([P, 16, NT], BF16)
                nc.sync.dma_start(base0_sb[:], b0_d[:])
                base123_sb = PD.tile([P, 16, NT], BF16)
                nc.sync.dma_start(base123_sb[:], b123_d[:])
                wpt_sb = PD.tile([P, 4, 4 * DH], BF16)
                nc.sync.dma_start(wpt_sb[:], _r(wpt))

                outs = PD.tile([P, 4, LMAX * NT], BF16)
                if DEC_STEPS != S or MERGED_STEPS != 3 * S:
                    nc.any.memset(outs[:], 0.0)
                cd = PD.tile([P, 4, Bc], F32)
                hd_bf = PD.tile([P, 4, Bc], BF16)
                nc.any.memset(cd[:], 0.0)
                nc.any.memset(hd_bf[:], 0.0)

                sigd = PD.tile([P, 16, Bc], F32)
                g1a_d = PD.tile([P, 16, Bc], F32)
                att_bf = PD.tile([S, Bc], BF16)
                rz_row = PD.tile([1, Bc], BF16)
                rzb_sb = PD.tile([P, 1, Bc], F32)
                A_t = PD.tile([P, 4, Bc], F32)
                B_t = PD.tile([P, 4, Bc], F32)
                tc_t = PD.tile([P, 4, Bc], F32)

                def dec_step(base_sb, base_off, outs_off, lv, tcx=None):
                    # scores, transposed: ps_scT[s, b] = sum_d whT[d,b,s]*hd[d,b]
                    ps_scT = PSD.tile([S, Bc], F32, tag="ps_sc")
                    for b in range(Bc):
                        for dt in range(4):
                            nc.tensor.matmul(
                                ps_scT[:, b:b + 1],
                                lhsT=whT[:, dt, b, :],
                                rhs=hd_bf[:, dt, b:b + 1],
                                start=(dt == 0), stop=(dt == 3),
                            )
                    # gates, hd half: can start immediately; per-level one-hot
                    # bias joins the accumulation as a K=1 matmul.
                    ps_g = PSD.tile([P, 16, Bc], F32, tag="ps_g")
                    for mt in range(16):
                        for kt in range(4):
                            nc.tensor.matmul(
                                ps_g[:, mt, :],
                                lhsT=wcdt_sb[:, kt, mt * P:(mt + 1) * P],
                                rhs=hd_bf[:, kt, :],
                                start=(kt == 0), stop=(kt == 3 and lv is None),
                            )
                        if lv is not None:
                            nc.tensor.matmul(
                                ps_g[:, mt, :],
                                lhsT=brow_sb[0:1, ds(lv * 2048 + mt * P, P)],
                                rhs=ones1[0:1, 0:Bc],
                                start=False, stop=True,
                            )
                    # softmax pieces (|scores| < ~1, no max-subtraction)
                    nc.scalar.activation(att_bf[:], ps_scT[:], AF.Exp)
                    ps_z = PSD.tile([1, Bc], F32, tag="ps_z")
                    nc.tensor.matmul(ps_z[:], lhsT=ones_col[:], rhs=att_bf[:],
                                     start=True, stop=True)
                    # 1/Z row immediately (critical: feeds the rzb broadcast)
                    with nc.allow_low_precision(reason="1/Z in bf16; att is bf16 anyway"):
                        nc.vector.reciprocal(rz_row[:], ps_z[:])
                    # gates, ctx half via G matvec (unnormalized)
                    ps_g2 = PSD.tile([P, 16, Bc], F32, tag="ps_g2")
                    for mt in range(16):
                        for b in range(Bc):
                            nc.tensor.matmul(
                                ps_g2[:, mt, b:b + 1],
                                lhsT=G_lay[:, b, mt * P:(mt + 1) * P],
                                rhs=att_bf[:, b:b + 1],
                                start=True, stop=True,
                            )
                    # 1/Z spread across partitions via K=1 ones matmul
                    ps_rzb = PSD.tile([P, 1, Bc], F32, tag="ps_rzb")
                    nc.tensor.matmul(ps_rzb[:, 0, :], lhsT=ones1[:],
                                     rhs=rz_row[:], start=True, stop=True)
                    nc.vector.tensor_copy(out=rzb_sb[:, 0, :], in_=ps_rzb[:, 0, :])
                    # hd-half + base folded early (off the critical path)
                    nc.vector.tensor_tensor(
                        out=g1a_d[:], in0=ps_g[:],
                        in1=base_sb[:, :, ds(base_off, Bc)], op=AX.add)
                    # g1 = g2*rz + g1a
                    g1_ps = PSD.tile([P, 16, Bc], F32, tag="g1_ps")
                    nc.vector.tensor_tensor(
                        out=g1_ps[:], in0=ps_g2[:],
                        in1=rzb_sb[:].to_broadcast([P, 16, Bc]), op=AX.mult)
                    nc.vector.tensor_tensor(
                        out=g1_ps[:], in0=g1_ps[:], in1=g1a_d[:], op=AX.add)
                    # doubled-basis cell: tau rows i(0:4) f(4:8) o(8:12) g(12:16)
                    nc.scalar.activation(sigd[:], g1_ps[:], AF.Tanh, scale=0.5)
                    nc.vector.scalar_tensor_tensor(
                        out=A_t[:], in0=sigd[:, 0:4, :], scalar=1.0,
                        in1=sigd[:, 12:16, :], op0=AX.add, op1=AX.mult)
                    nc.vector.scalar_tensor_tensor(
                        out=B_t[:], in0=sigd[:, 4:8, :], scalar=1.0,
                        in1=cd[:], op0=AX.add, op1=AX.mult)
                    nc.vector.scalar_tensor_tensor(
                        out=cd[:], in0=B_t[:], scalar=0.5,
                        in1=A_t[:], op0=AX.mult, op1=AX.add)
                    nc.scalar.activation(tc_t[:], cd[:], AF.Tanh, scale=0.5)
                    nc.vector.scalar_tensor_tensor(
                        out=hd_bf[:], in0=sigd[:, 8:12, :], scalar=1.0,
                        in1=tc_t[:], op0=AX.add, op1=AX.mult)
                    nc.scalar.copy(out=outs[:, :, ds(outs_off, Bc)], in_=hd_bf[:])

                # level 0
                DU = 2 if DEC_STEPS % 2 == 0 and DEC_STEPS > 0 else 1
                with tc.For_i(0, DEC_STEPS // DU, hint_engines=(mybir.EngineType.PE,), staggered_reset=True) as i:
                    for u in range(DU):
                        off0 = i * (DU * Bc) + u * Bc
                        if u > 0:
                            tc.stage_boundary()
                        dec_step(base0_sb, off0, off0, None, tc)
                    tc.stage_boundary()
                    tc.stage_boundary()

                # fold W_p @ outs[level 0] into base123
                for mt in range(16):
                    for nch in range(2):
                        pst = PSB.tile([P, 512], F32, tag="psbig")
                        for kt in range(4):
                            nc.tensor.matmul(
                                pst[:],
                                lhsT=wpt_sb[:, kt, mt * P:(mt + 1) * P],
                                rhs=outs[:, kt, nch * 512:(nch + 1) * 512],
                                start=(kt == 0), stop=(kt == 3),
                            )
                        bslice = base123_sb[:, mt, nch * 512:(nch + 1) * 512]
                        nc.vector.tensor_tensor(
                            out=bslice, in0=bslice, in1=pst[:], op=AX.add)

                # levels 1..3: separate loops (stationary operands need
                # static offsets, so the level index must be a constant)
                for lv in (1, 2, 3):
                    n_steps = min(S, max(0, MERGED_STEPS - (lv - 1) * S))
                    if n_steps == 0:
                        continue
                    LU = 2 if n_steps % 2 == 0 else 1
                    with tc.For_i(0, n_steps // LU, hint_engines=(mybir.EngineType.PE,), staggered_reset=True) as i:
                        for u in range(LU):
                            toff = i * (LU * Bc) + u * Bc
                            if u > 0:
                                tc.stage_boundary()
                            dec_step(base123_sb, toff, i * (LU * Bc) + (lv * NT + u * Bc), lv, tc)
                        tc.stage_boundary()
                        tc.stage_boundary()

                # ---------------- phase 6: logits ----------------
                for lvl in range(LMAX):
                    lg = PDS.tile([C, NT], F32, tag="lg")
                    for nch in range(2):
                        ps_lg = PSB.tile([P, 512], F32, tag="psbig")
                        for kt in range(4):
                            nc.tensor.matmul(
                                ps_lg[:C, :],
                                lhsT=w2t_sb[:, kt, :],
                                rhs=outs[:, kt,
                                         lvl * NT + nch * 512:lvl * NT + (nch + 1) * 512],
                                start=(kt == 0), stop=(kt == 3),
                            )
                        nc.vector.tensor_tensor(
                            out=lg[:, nch * 512:(nch + 1) * 512],
                            in0=ps_lg[:C, :],
                            in1=b2_sb[:].to_broadcast([C, 512]),
                            op=AX.add,
                        )
                    nc.sync.dma_start(out[lvl], lg[:])

    _split_sync_waits(nc, max_waits=1)
    return nc


# gate blocks (of the 4*Hd output dim): pytorch order i,f,g,o -> i,f,o,g
def _perm_gates(w, hd, axis):
    """Reorder gate blocks i,f,g,o -> i,f,o,g along `axis` (block size hd)."""
    blocks = np.split(np.asarray(w), 4, axis=axis)
    return np.concatenate([blocks[0], blocks[1], blocks[3], blocks[2]], axis=axis)


def _gscale(w, hd, axis, g_mult, other_mult=1.0):
    """Scale the (already permuted) g block (last) by g_mult, rest by other."""
    w = np.array(w, dtype=np.float32, copy=True)
    sl = [slice(None)] * w.ndim
    sl[axis] = slice(0, 3 * hd)
    w[tuple(sl)] *= other_mult
    sl[axis] = slice(3 * hd, 4 * hd)
    w[tuple(sl)] *= g_mult
    return w


def host_prep(inputs):
    """Build the per-core in_maps from the full problem inputs.

    Doubled basis: device h/hd carry 2x the real values, so weights that
    consume them are scaled by 0.5 (Wl by 0.25: both operands doubled).
    Tanh-gate trick: g-gate output rows are scaled by 2 so one tanh(x/2)
    activation yields tanh(pre) for g and tanh(pre/2) for sigmoid gates.
    """
    f32 = lambda a: np.ascontiguousarray(np.asarray(a, dtype=np.float32))
    bf16 = lambda a: np.ascontiguousarray(
        np.asarray(a, dtype=np.float32).astype(ml_dtypes.bfloat16))

    seqs = np.asarray(inputs["seqs"])
    emb = f32(inputs["emb"])

    # encoder weights: reorder i,f,g,o -> i,f,o,g; g rows x2; Whh x0.5 (h dbl)
    def enc_prep(wih, whh, bih, bhh):
        wih = _gscale(_perm_gates(f32(inputs[wih]), H, 0), H, 0, 2.0, 1.0)
        whh = _gscale(_perm_gates(f32(inputs[whh]), H, 0), H, 0, 1.0, 0.5)
        bias = _gscale(_perm_gates(
            f32(inputs[bih]) + f32(inputs[bhh]), H, 0), H, 0, 2.0, 1.0)
        return wih.T.copy(), whh.T.copy(), bias

    wihf_t, whhf_t, bf_ = enc_prep("Wih_f", "Whh_f", "bih_f", "bhh_f")
    wihb_t, whhb_t, bb_ = enc_prep("Wih_b", "Whh_b", "bih_b", "bhh_b")
    benc = np.stack([bf_.reshape(8, P).T, bb_.reshape(8, P).T], axis=1)

    wl_t = f32(inputs["Wl"]).T * 0.25

    wih_d = f32(inputs["Wih_d"])
    whh_d = f32(inputs["Whh_d"])
    bd = f32(inputs["bih_d"]) + f32(inputs["bhh_d"])
    w_ctx = wih_d[:, 0:DH]
    w_h = wih_d[:, DH:2 * DH]
    w_e = wih_d[:, 2 * DH:3 * DH]
    w_p = wih_d[:, 3 * DH:4 * DH]
    w_oh = wih_d[:, 4 * DH:4 * DH + LMAX]

    # decoder matrices: [4*DH(out), K] -> perm+gscale along axis 0, then .T
    def dec_w(w, in_mult):
        return _gscale(_perm_gates(w, DH, 0), DH, 0, 2.0 * in_mult, in_mult).T.copy()

    wcd_t = dec_w(whh_d, 0.5)                                      # [512, 2048]
    wct_t = dec_w(w_ctx, 0.5)                                      # [512, 2048]
    wa_t = np.concatenate(
        [dec_w(w_h + w_p, 0.5), dec_w(w_e, 1.0)], axis=0)          # [1024, 2048]
    wb_t = np.concatenate(
        [dec_w(w_h, 0.5), dec_w(w_e, 1.0)], axis=0)                # [1024, 2048]
    wp_t = dec_w(w_p, 0.5)                                         # [512, 2048]

    bias_l = bd[None, :] + w_oh.T                                  # [4, 2048]
    bias_l = _gscale(_perm_gates(bias_l, DH, 1), DH, 1, 2.0, 1.0)
    bcols = bias_l.T.copy()                                        # [2048, 4]
    bdec = bcols.reshape(16, P, 4).transpose(1, 0, 2).copy()       # [p, mt, col]
    brow = bias_l.reshape(1, LMAX * 4 * DH)

    w2_t = f32(inputs["W2"]).T * 0.5
    b2v = f32(inputs["b2"]).reshape(C, 1)

    shared = {
        "emb": emb,
        "wihf": bf16(wihf_t), "wihb": bf16(wihb_t),
        "whhf": bf16(whhf_t), "whhb": bf16(whhb_t),
        "benc": f32(benc),
        "wlt": bf16(wl_t),
        "wcdt": bf16(wcd_t), "wct": bf16(wct_t),
        "wat": bf16(wa_t), "wbt": bf16(wb_t),
        "wpt": bf16(wp_t),
        "bdec": f32(bdec), "brow": bf16(brow),
        "w2t": bf16(w2_t),
        "b2v": b2v,
    }
    in_maps = []
    for c in range(NCORES):
        m = dict(shared)
        m["idx"] = np.ascontiguousarray(
            seqs[c * Bc:(c + 1) * Bc].T.astype(np.uint32))          # [S, Bc]
        in_maps.append(m)
    return in_maps


_NC_CACHE = {}


def get_nc(debug=False):
    if debug not in _NC_CACHE:
        _NC_CACHE[debug] = build_nc(debug)
    return _NC_CACHE[debug]


def kernel(**inputs):
    from concourse.bass_utils import run_bass_kernel_spmd

    nc = get_nc(debug=False)
    in_maps = host_prep(inputs)
    res = run_bass_kernel_spmd(nc, in_maps, core_ids=list(range(NCORES)))
    lvl = int(np.asarray(inputs["seq_max_nested_level"]))
    lvl = max(1, min(LMAX, lvl))
    # out per core: [LMAX, C, NT] with token = t*Bc + b
    full = np.empty((LMAX, S, B, C), dtype=np.float32)
    for c in range(NCORES):
        o = np.asarray(res.results[c]["out"])
        full[:, :, c * Bc:(c + 1) * Bc, :] = (
            o.transpose(0, 2, 1).reshape(LMAX, S, Bc, C))
    return full[:lvl].reshape(-1, C)
